# revision 6
# baseline (speedup 1.0000x reference)
"""Trainium2 Bass kernel: batched Sinkhorn-Knopp OT loss (nn_CTR_12232066859248).

Reference semantics (B=4096 batch rows, K=128 bins):
    Kmat = exp(-M * 20)
    u0 = 1/K; repeat: v = b / (Kmat^T u); u = a / (Kmat v)
    early-exit check every 50 iters (at cpt=1, 51): err = max_b sum_k |v*(Kmat^T u) - b|
    stop when err <= 0.005 or cpt == 100
    loss = mean_b u^T (Kmat*M) v

Sharding: data-parallel over B across 8 cores (512 rows each); Kmat replicated.
Everything on-chip lives in transposed layout [K=128 partitions, batch in free dim]
so both matmuls contract over the partition dim with no transposes in the loop.

The data-dependent trip count (1, 51, or 100 iterations) is handled on the host:
one NEFF runs 51 iterations and emits err/loss at checkpoints 1 and 51; the host
applies the reference's exit logic to the gathered scalars. Only if the data has
not converged by iteration 51 (never the case for the shipped inputs) is a
second 100-iteration NEFF compiled and run.
"""

import os
import sys

import numpy as np

for _p in ("/opt/trn_rl_repo", "/root/.axon_site/_ro/trn_rl_repo"):
    if os.path.isdir(_p) and _p not in sys.path:
        sys.path.insert(0, _p)
        break

from contextlib import ExitStack

import concourse.bass as bass
import concourse.mybir as mybir
import concourse.tile as tile
from concourse import bacc
from concourse.bass_utils import run_bass_kernel_spmd

B, K = 4096, 128
N_CORES = 8
BS = B // N_CORES  # 512 batch rows per core
ALPHA = 20.0
THR = 0.005
F32 = mybir.dt.float32
BF16 = mybir.dt.bfloat16
AX = mybir.AxisListType
ALU = mybir.AluOpType
ACT_FN = mybir.ActivationFunctionType

_NC_CACHE: dict = {}


def _emit_err(nc, pools, km, u, v, b_sb, ones, out_d):
    """err = max_b sum_j |v[j,b] * (Kmat^T u)[j,b] - b[j,b]| -> [1,1] scalar."""
    tmp, psum, psred = pools["tmp"], pools["psum"], pools["psred"]
    psc = psum.tile([K, BS], F32, tag="ps", name=f"psc_{out_d.name}")
    nc.tensor.matmul(psc[:], km[:], u[:])
    bb = tmp.tile([K, BS], F32, tag="chk", name=f"bb_{out_d.name}")
    nc.vector.tensor_mul(bb[:], v[:], psc[:])
    d = tmp.tile([K, BS], F32, tag="chk", name=f"d_{out_d.name}")
    nc.vector.tensor_sub(d[:], bb[:], b_sb[:])
    dabs = tmp.tile([K, BS], F32, tag="chk", name=f"dabs_{out_d.name}")
    nc.scalar.activation(dabs[:], d[:], ACT_FN.Abs)
    pr = psred.tile([1, BS], F32, tag="red", name=f"pr_{out_d.name}")
    nc.tensor.matmul(pr[:], ones[:], dabs[:])  # column sums -> per-row L1
    sc = tmp.tile([1, 1], F32, tag="sc", name=f"esc_{out_d.name}")
    nc.vector.tensor_reduce(sc[:], pr[:], axis=AX.X, op=ALU.max)
    nc.sync.dma_start(out_d, sc[:])


def _emit_loss(nc, pools, kmmT, u, v, ones, out_d):
    """partial loss = sum_b u[:,b]^T (Kmat*M) v[:,b] -> [1,1] scalar."""
    tmp, psum, psred = pools["tmp"], pools["psum"], pools["psred"]
    pl = psum.tile([K, BS], F32, tag="ps", name=f"pl_{out_d.name}")
    nc.tensor.matmul(pl[:], kmmT[:], v[:])  # [i,b] = sum_j (Kmat*M)[i,j] v[j,b]
    z = tmp.tile([K, BS], F32, tag="chk", name=f"z_{out_d.name}")
    nc.vector.tensor_mul(z[:], u[:], pl[:])
    pr = psred.tile([1, BS], F32, tag="red", name=f"prl_{out_d.name}")
    nc.tensor.matmul(pr[:], ones[:], z[:])
    sc = tmp.tile([1, 1], F32, tag="sc", name=f"lsc_{out_d.name}")
    nc.vector.tensor_reduce(sc[:], pr[:], axis=AX.X, op=ALU.add)
    nc.sync.dma_start(out_d, sc[:])


def _build(n_iters: int, checkpoints: tuple[int, ...]):
    """One NEFF: n_iters Sinkhorn iterations; at each checkpoint t emit err{t}
    and loss{t}; always emit loss{n_iters} at the end."""
    nc = bacc.Bacc(
        "TRN2", target_bir_lowering=False, debug=False, num_devices=N_CORES
    )
    aT_d = nc.dram_tensor("at_in", [K, BS], F32, kind="ExternalInput").ap()
    bT_d = nc.dram_tensor("bt_in", [K, BS], F32, kind="ExternalInput").ap()
    m_d = nc.dram_tensor("m_in", [K, K], F32, kind="ExternalInput").ap()
    mT_d = nc.dram_tensor("mt_in", [K, K], F32, kind="ExternalInput").ap()

    out_names = []
    for t in checkpoints:
        out_names += [f"err{t}", f"loss{t}"]
    if f"loss{n_iters}" not in out_names:
        out_names.append(f"loss{n_iters}")
    outs_d = {
        n: nc.dram_tensor(n, [1, 1], F32, kind="ExternalOutput").ap()
        for n in out_names
    }

    with tile.TileContext(nc) as tc, ExitStack() as ctx:
        const = ctx.enter_context(tc.tile_pool(name="const", bufs=1))
        state = ctx.enter_context(tc.tile_pool(name="state", bufs=3))
        tmp = ctx.enter_context(tc.tile_pool(name="tmp", bufs=3))
        psum = ctx.enter_context(tc.tile_pool(name="psum", bufs=4, space="PSUM"))
        psred = ctx.enter_context(tc.tile_pool(name="psred", bufs=2, space="PSUM"))
        pools = {"tmp": tmp, "psum": psum, "psred": psred}

        m_sb = const.tile([K, K], F32)
        nc.sync.dma_start(m_sb[:], m_d)
        mT_sb = const.tile([K, K], F32)
        nc.sync.dma_start(mT_sb[:], mT_d)
        a_sb = const.tile([K, BS], F32)
        nc.sync.dma_start(a_sb[:], aT_d)
        b_sb = const.tile([K, BS], F32)
        nc.sync.dma_start(b_sb[:], bT_d)

        km = const.tile([K, K], BF16)  # Kmat, layout [k, j]
        nc.scalar.activation(km[:], m_sb[:], ACT_FN.Exp, scale=-ALPHA)
        kmT = const.tile([K, K], BF16)  # Kmat^T, layout [j, k]
        nc.scalar.activation(kmT[:], mT_sb[:], ACT_FN.Exp, scale=-ALPHA)
        kmmT = const.tile([K, K], BF16)  # (Kmat*M)^T for the loss matmul
        nc.vector.tensor_mul(kmmT[:], kmT[:], mT_sb[:])
        ones = const.tile([K, 1], F32)
        nc.vector.memset(ones[:], 1.0)

        u = state.tile([K, BS], BF16, tag="u", name="u_init")
        nc.vector.memset(u[:], 1.0 / K)
        v = None
        for t in range(1, n_iters + 1):
            ps1 = psum.tile([K, BS], F32, tag="ps", name=f"ps1_{t}")
            nc.tensor.matmul(ps1[:], km[:], u[:])
            r1 = tmp.tile([K, BS], F32, tag="r", name=f"r1_{t}")
            nc.vector.reciprocal_approx_fast(r1[:], ps1[:])
            v = state.tile([K, BS], BF16, tag="v", name=f"v_{t}")
            nc.vector.tensor_mul(v[:], b_sb[:], r1[:])
            ps2 = psum.tile([K, BS], F32, tag="ps", name=f"ps2_{t}")
            nc.tensor.matmul(ps2[:], kmT[:], v[:])
            r2 = tmp.tile([K, BS], F32, tag="r", name=f"r2_{t}")
            nc.vector.reciprocal_approx_fast(r2[:], ps2[:])
            u_new = state.tile([K, BS], BF16, tag="u", name=f"u_{t}")
            nc.vector.tensor_mul(u_new[:], a_sb[:], r2[:])
            u = u_new

            if t in checkpoints:
                _emit_err(nc, pools, km, u, v, b_sb, ones, outs_d[f"err{t}"])
            if t in checkpoints or t == n_iters:
                _emit_loss(nc, pools, kmmT, u, v, ones, outs_d[f"loss{t}"])

    nc.compile()
    return nc


def _get_nc(key):
    if key not in _NC_CACHE:
        n_iters, checkpoints = key
        _NC_CACHE[key] = _build(n_iters, checkpoints)
    return _NC_CACHE[key]


def _make_in_maps(a, b, M):
    aT = np.ascontiguousarray(a.T.astype(np.float32, copy=False))  # [K, B]
    bT = np.ascontiguousarray(b.T.astype(np.float32, copy=False))
    M = np.ascontiguousarray(M.astype(np.float32, copy=False))
    MT = np.ascontiguousarray(M.T)
    return [
        {
            "at_in": np.ascontiguousarray(aT[:, i * BS : (i + 1) * BS]),
            "bt_in": np.ascontiguousarray(bT[:, i * BS : (i + 1) * BS]),
            "m_in": M,
            "mt_in": MT,
        }
        for i in range(N_CORES)
    ]


def _run(nc, in_maps, _collect=None, **kwargs):
    out = run_bass_kernel_spmd(nc, in_maps, list(range(N_CORES)), **kwargs)
    if _collect is not None:
        _collect.append(out)
    return out.results


def kernel(a, b, M, _collect=None, **run_kwargs):
    """Full-input entry point: a, b (4096,128) f32; M (128,128) f32 -> scalar f32."""
    in_maps = _make_in_maps(a, b, M)

    nc = _get_nc((51, (1, 51)))
    res = _run(nc, in_maps, _collect=_collect, **run_kwargs)

    def gather(name, reduce_fn):
        return reduce_fn([float(r[name][0, 0]) for r in res])

    # Mirror the reference's while-loop exit logic on the per-checkpoint
    # global scalars. err is the max over all batch rows (= max over cores).
    if gather("err1", max) <= THR:
        total = gather("loss1", sum)
    elif gather("err51", max) <= THR:
        total = gather("loss51", sum)
    else:
        # Not converged by 51: the reference runs the full 100 iterations
        # (no further checks fire before cpt==100). Rare path, compiled lazily.
        nc2 = _get_nc((100, ()))
        res2 = _run(nc2, in_maps, _collect=_collect, **run_kwargs)
        total = sum(float(r["loss100"][0, 0]) for r in res2)

    return np.float32(total / B)
